# revision 31
# baseline (speedup 1.0000x reference)
"""NVFP4-fake-quant MLP (x@w1.T -> gelu -> @w2.T) on 8 trn2 NeuronCores.

Sharding (megatron tensor-parallel on the hidden dim):
  core c holds w1 rows [c*2048:(c+1)*2048], w2 cols [c*2048:(c+1)*2048],
  and x rows [c*1024:(c+1)*1024] (for distributed x-quantization).

Exact quantization:
  per-16-block e4m3 scales via exponent-mask + magic-number RNE;
  fp4 e2m1 rounding via 3-region clamp + magic-round decomposition.
  e2m1_value * e4m3_blockscale has <= 6 mantissa bits -> stored EXACTLY in
  bf16, so the bf16 matmuls reproduce the f32 reference; per-tensor scales
  are folded into the PSUM->SBUF copies (gelu input scale / output scale).

v5 schedule -- quant calls run PAIRED (software-pipelined across the
Vector/Scalar ping-pong), PE starts ~0.6ms in and stays dense:
  scans->AR1 first; w1 quantizes hi-half (rows 1024-2047) then lo-half,
  each into a 64KB w1T window (never 128KB resident).  x quantizes in
  row-halves with per-half xqT AllGathers.  M1 = two 1024-wide column
  sweeps (hi then lo); gelu drains spill g to DRAM f32.  w2 amax/AR2 +
  full w2 quant ride Vector slack inside the sweeps.  At M1 end the w1T
  window is swapped for right-side w2T quarter pools; M2 is one global
  b-tile pass; h-quant pairs chase M2 two tiles ahead, hq stays in SBUF
  and is transposed SBUF->SBUF straight into the stationary hqT tiles.
  ReduceScatter fires per 512-row chunk; f32 cast-stores chase it.
DMA placement: ACT ring = dep-free loads + x-side stores/transposes +
  g stores (self-paced by ACT compute); SP ring = weight stores, w1T/w2T
  transposes, xb loads, parts stores; SWDGE = collectives + AR staging +
  output casts.  Emission order per ring matches execution order.
"""
import os
import sys
import numpy as np

if "/opt/trn_rl_repo" not in sys.path:
    sys.path.insert(0, "/opt/trn_rl_repo")

f32 = np.float32

B, D_IN, HID, D_OUT = 8192, 4096, 16384, 4096
NCORES = 8
BSH = B // NCORES          # 1024 x-rows quantized per core
HSH = HID // NCORES        # 2048 hidden units per core
NBT = B // 128             # 64 b-tiles
RSCH = 16                  # reduce-scatter chunks
RSROWS = B // RSCH         # 512 rows per RS chunk
RSOUT = RSROWS // NCORES   # 64 rows per core per chunk
NK1 = D_IN // 128          # 32 k-tiles, first matmul
NK2 = HSH // 128           # 16 k-tiles, second matmul

# magic round-to-nearest-even constants (f32-exact)
C_HALF = float(f32(1.5 * 2 ** 22))       # grid 0.5
C_1 = float(f32(1.5 * 2 ** 23))          # grid 1
C_1B = float(f32(1.5 * 2 ** 23 + 2.0))   # C_1 + 2
C_2 = float(f32(1.5 * 2 ** 24))          # grid 2
C_2B = float(f32(1.5 * 2 ** 24 + 4.0))   # C_2 + 4
E4M3_MAGIC = float(f32(1.5 * 2 ** 20))   # * 2^e -> magic const for step 2^(e-3)
EXPMASK = 0x7F800000
SIGNMASK = 0x80000000
ONEBITS = 0x3F800000

_BUILT = {}


def _build(isc, hsc):
    from contextlib import ExitStack
    import concourse.bass as bass
    import concourse.tile as tile
    from concourse import bacc, mybir

    OP = mybir.AluOpType
    AF = mybir.ActivationFunctionType
    U32 = mybir.dt.uint32
    FP32 = mybir.dt.float32
    BF16 = mybir.dt.bfloat16

    c1x = float(f32(1.0) / (f32(6.0) * f32(isc)))
    c1h = float(f32(1.0) / (f32(6.0) * f32(hsc)))
    inv2688 = float(f32(1.0) / f32(2688.0))
    RG = [list(range(NCORES))]

    nc = bacc.Bacc("TRN2", target_bir_lowering=False, debug=False,
                   num_devices=NCORES)
    x_sh = nc.dram_tensor("x_sh", [BSH, D_IN], FP32, kind="ExternalInput").ap()
    w1_sh = nc.dram_tensor("w1_sh", [HSH, D_IN], FP32, kind="ExternalInput").ap()
    w2_sh = nc.dram_tensor("w2_sh", [D_OUT, HSH], FP32, kind="ExternalInput").ap()
    out_sh = nc.dram_tensor("out_sh", [BSH, D_OUT], FP32, kind="ExternalOutput").ap()

    with tile.TileContext(nc) as tc, ExitStack() as top:
        dram = top.enter_context(tc.tile_pool(name="dram", bufs=1, space="DRAM"))
        amax_stage = dram.tile([128, 2], FP32, tag="amax_stage", name="amax_stage")
        s1loc = dram.tile([1, 1], FP32, tag="s1loc", name="s1loc")
        s2loc = dram.tile([1, 1], FP32, tag="s2loc", name="s2loc")
        s1sh = dram.tile([1, 1], FP32, tag="s1sh", name="s1sh", addr_space="Shared")
        s2sh = dram.tile([1, 1], FP32, tag="s2sh", name="s2sh", addr_space="Shared")
        xq_loc = dram.tile([BSH, D_IN], BF16, tag="xq_loc", name="xq_loc")
        xqT_locs = [dram.tile([D_IN, 512], BF16, tag=f"xqT_loc{h}",
                              name=f"xqT_loc{h}") for h in range(2)]
        xqT_fulls = [dram.tile([NCORES * D_IN, 512], BF16,
                               tag=f"xqT_full{h}", name=f"xqT_full{h}",
                               addr_space="Shared") for h in range(2)]
        w1q = dram.tile([HSH, D_IN], BF16, tag="w1q", name="w1q")
        w2q = dram.tile([D_OUT, HSH], BF16, tag="w2q", name="w2q")
        g_dram = dram.tile([B, HSH], FP32, tag="g_dram", name="g_dram")
        parts = [dram.tile([RSROWS, D_OUT], BF16, name=f"part{c}", tag=f"part{c}")
                 for c in range(RSCH)]
        rsouts = [dram.tile([RSOUT, D_OUT], BF16, name=f"rsout{c}",
                            tag=f"rsout{c}")
                  for c in range(RSCH)]

        singles = top.enter_context(tc.tile_pool(name="singles", bufs=1))
        biases = {}
        for nm, val in [("ch", C_HALF), ("nch", -C_HALF),
                        ("c1", C_1), ("nc1b", -C_1B),
                        ("c2", C_2), ("nc2b", -C_2B)]:
            bt = singles.tile([128, 1], FP32, tag=f"bias_{nm}", name=f"bias_{nm}")
            nc.vector.memset(bt[:], val)
            biases[nm] = bt
        acc1 = singles.tile([128, 1], FP32, tag="acc1", name="acc1")
        acc2 = singles.tile([128, 1], FP32, tag="acc2", name="acc2")
        tsw1 = singles.tile([128, 1], FP32, tag="tsw1", name="tsw1")
        rdw1 = singles.tile([128, 1], FP32, tag="rdw1", name="rdw1")
        s_h = singles.tile([128, 1], FP32, tag="s_h", name="s_h")
        tsw2 = singles.tile([128, 1], FP32, tag="tsw2", name="tsw2")
        rdw2 = singles.tile([128, 1], FP32, tag="rdw2", name="rdw2")
        s_o = singles.tile([128, 1], FP32, tag="s_o", name="s_o")

        qp_src = top.enter_context(tc.tile_pool(name="qp_src", bufs=3))
        qp_f = top.enter_context(tc.tile_pool(name="qp_f", bufs=2))
        qp_b = top.enter_context(tc.tile_pool(name="qp_b", bufs=2))
        qp_n = top.enter_context(tc.tile_pool(name="qp_n", bufs=2))
        hqp = top.enter_context(tc.tile_pool(name="hqp", bufs=4))
        # m1x/m1g are released at M1 end (before w2T quarters 2-4 allocate)
        m1x_cm = tc.tile_pool(name="m1x", bufs=3)
        m1x = m1x_cm.__enter__()
        m1g_cm = tc.tile_pool(name="m1g", bufs=3)
        m1g = m1g_cm.__enter__()

        # ---------------- paired quant pipeline ----------------
        W = 1024
        NB = W // 16

        def q_load(src_slice):
            st = qp_src.tile([128, W], FP32, tag="xt", name="q_in")
            nc.scalar.dma_start(st[:], src_slice)
            return st

        def q_head(st, c1, effmul, signed):
            s = {"src": st, "signed": signed}
            if signed:
                absv = qp_f.tile([128, W], FP32, tag="q_absv", name="q_absv")
                nc.scalar.activation(absv[:], st[:], AF.Abs)
                s["mag"] = absv
            else:
                s["mag"] = st
            amax = qp_n.tile([128, NB], FP32, tag="q_amax", name="q_amax")
            nc.vector.tensor_reduce(
                amax[:], s["mag"][:].rearrange("p (nb b) -> p nb b", b=16),
                axis=mybir.AxisListType.X, op=OP.max,
                apply_absolute_value=(None if signed else True))
            vq = qp_n.tile([128, NB], FP32, tag="q_vq", name="q_vq")
            nc.vector.tensor_scalar(vq[:], amax[:], c1, None, OP.mult)
            scq = qp_n.tile([128, NB], FP32, tag="q_amax", name="q_scq")
            nc.vector.tensor_scalar(scq[:].bitcast(U32), vq[:].bitcast(U32),
                                    EXPMASK, None, OP.bitwise_and)
            cb = qp_n.tile([128, NB], FP32, tag="q_cb", name="q_cb")
            nc.vector.tensor_scalar(cb[:], scq[:], E4M3_MAGIC, None, OP.mult)
            t4 = qp_n.tile([128, NB], FP32, tag="q_t4", name="q_t4")
            nc.vector.tensor_tensor(t4[:], vq[:], cb[:], OP.add)
            bs = qp_n.tile([128, NB], FP32, tag="q_vq", name="q_bs")
            nc.vector.tensor_tensor(bs[:], t4[:], cb[:], OP.subtract)
            bs16 = qp_n.tile([128, NB], BF16, tag="q_bs16", name="q_bs16")
            nc.vector.tensor_scalar(bs16[:], bs[:], 2.0 ** -6, None, OP.max)
            eff = qp_n.tile([128, NB], FP32, tag="q_t4", name="q_eff")
            nc.vector.tensor_scalar(eff[:], bs[:], 2.0 ** -6, effmul,
                                    OP.max, OP.mult)
            rec = qp_n.tile([128, NB], FP32, tag="q_rec", name="q_rec")
            nc.vector.reciprocal(rec[:], eff[:])
            r = qp_f.tile([128, W], FP32, tag="q_r", name="q_r")
            nc.vector.tensor_tensor(
                r[:].rearrange("p (nb b) -> p nb b", b=16),
                s["mag"][:].rearrange("p (nb b) -> p nb b", b=16),
                rec[:, :, None].to_broadcast([128, NB, 16]), OP.mult)
            m1 = qp_f.tile([128, W], FP32, tag="q_absv", name="q_m1")
            nc.vector.tensor_scalar(m1[:], r[:], 2.0, None, OP.min)
            m3 = qp_f.tile([128, W], FP32, tag="q_m23", name="q_m3")
            nc.vector.tensor_scalar(m3[:], r[:], 4.0, 6.0, OP.max, OP.min)
            m2 = qp_f.tile([128, W], FP32, tag="q_m23", name="q_m2")
            nc.vector.tensor_scalar(m2[:], r[:], 2.0, 4.0, OP.max, OP.min)
            s.update(bs16=bs16, r=r, m1=m1, m2=m2, m3=m3)
            return s

        def q_acts(s):
            m1, m2, m3 = s["m1"], s["m2"], s["m3"]
            nc.scalar.activation(m3[:], m3[:], AF.Identity, bias=biases["c2"][:])
            s3 = qp_b.tile([128, W], BF16, tag="q_s3", name="q_s3")
            nc.scalar.activation(s3[:], m3[:], AF.Identity, bias=biases["nc2b"][:])
            nc.scalar.activation(m2[:], m2[:], AF.Identity, bias=biases["c1"][:])
            s2 = qp_b.tile([128, W], BF16, tag="q_s2", name="q_s2")
            nc.scalar.activation(s2[:], m2[:], AF.Identity, bias=biases["nc1b"][:])
            nc.scalar.activation(m1[:], m1[:], AF.Identity, bias=biases["ch"][:])
            s1 = qp_b.tile([128, W], BF16, tag="q_s1", name="q_s1", bufs=2)
            nc.scalar.activation(s1[:], m1[:], AF.Identity, bias=biases["nch"][:])
            s.update(s1=s1, s2=s2, s3=s3)

        def q_tail(s, out_ap):
            q12 = qp_b.tile([128, W], BF16, tag="q_q12", name="q_q12", bufs=2)
            nc.vector.tensor_tensor(q12[:], s["s1"][:], s["s2"][:], OP.add)
            qq = qp_b.tile([128, W], BF16, tag="q_s2", name="q_qq")
            nc.vector.tensor_tensor(qq[:], q12[:], s["s3"][:], OP.add)
            bs16 = s["bs16"]
            if s["signed"]:
                qs = qp_b.tile([128, W], BF16, tag="q_s1", name="q_qs", bufs=2)
                nc.vector.tensor_tensor(
                    qs[:].rearrange("p (nb b) -> p nb b", b=16),
                    qq[:].rearrange("p (nb b) -> p nb b", b=16),
                    bs16[:, :, None].to_broadcast([128, NB, 16]), OP.mult)
                sgn = qp_f.tile([128, W], FP32, tag="q_r", name="q_sgn")
                nc.vector.tensor_scalar(sgn[:].bitcast(U32),
                                        s["src"][:].bitcast(U32),
                                        SIGNMASK, ONEBITS,
                                        OP.bitwise_and, OP.bitwise_or)
                nc.vector.tensor_tensor(out_ap, qs[:], sgn[:], OP.mult)
            else:
                nc.vector.tensor_tensor(
                    out_ap.rearrange("p (nb b) -> p nb b", b=16),
                    qq[:].rearrange("p (nb b) -> p nb b", b=16),
                    bs16[:, :, None].to_broadcast([128, NB, 16]), OP.mult)

        def quant_pair(srcs, dsts, c1, effmul, signed=True, store_eng=None):
            """Two software-pipelined quant calls.  srcs: 2 DRAM slices;
            dsts: 2 DRAM slices (stored via store_eng) or None (returns
            the xo SBUF tiles)."""
            sts = [q_load(sl) for sl in srcs]
            states = [q_head(st, c1, effmul, signed) for st in sts]
            for s in states:
                q_acts(s)
            outs = []
            for i, s in enumerate(states):
                ot = qp_src.tile([128, W], BF16, tag="xo", name="q_out", bufs=2)
                q_tail(s, ot[:])
                if dsts is not None:
                    store_eng.dma_start(dsts[i], ot[:])
                outs.append(ot)
            return outs

        def scan_chunk(w_ap, i, acc, ncc):
            wt = qp_src.tile([128, W], FP32, tag="xt", name="scan_in")
            nc.scalar.dma_start(
                wt[:], w_ap[(i // ncc) * 128:(i // ncc + 1) * 128,
                            (i % ncc) * W:(i % ncc + 1) * W])
            am = qp_n.tile([128, 1], FP32, tag="am_w", name="am_w")
            nc.vector.tensor_reduce(am[:], wt[:], axis=mybir.AxisListType.X,
                                    op=OP.max, apply_absolute_value=True)
            if i == 0:
                nc.vector.tensor_copy(acc[:], am[:])
            else:
                nc.vector.tensor_tensor(acc[:], acc[:], am[:], OP.max)

        def allreduce_amax(acc, col, loc, sh, sam_name):
            # staging rides the SWDGE ring (gpsimd) so it never queues
            # behind PE-paced SP traffic
            nc.gpsimd.dma_start(amax_stage[:, col:col + 1], acc[:])
            rowv = singles.tile([1, 128], FP32, tag=f"rowv{col}",
                                name=f"rowv{col}")
            nc.gpsimd.dma_start(
                rowv[:],
                amax_stage[:, col:col + 1].rearrange("p c -> (p c)").unsqueeze(0))
            red = singles.tile([1, 1], FP32, tag=f"red{col}", name=f"red{col}")
            nc.vector.tensor_reduce(red[:], rowv[:],
                                    axis=mybir.AxisListType.X, op=OP.max)
            nc.gpsimd.dma_start(loc[:], red[:])
            nc.gpsimd.collective_compute(
                "AllReduce", OP.max, replica_groups=RG,
                ins=[loc[:].opt()], outs=[sh[:].opt()])
            sam = singles.tile([128, 1], FP32, tag=sam_name, name=sam_name)
            ap = sh[:]
            nc.gpsimd.dma_start(sam[:], bass.AP(
                tensor=ap.tensor, offset=ap.offset,
                ap=[[0, 128]] + list(ap.ap)[1:]))
            return sam

        # ---------------- phase-0 building blocks ----------------
        def x_half_quant(h):
            # quantize my x rows [h*512:(h+1)*512]; stores + transposes on
            # the ACT ring (self-paced with the quant chain)
            for i in range(h * 4, h * 4 + 4):
                for cp in range(2):
                    c0 = cp * 2
                    quant_pair(
                        [x_sh[i * 128:(i + 1) * 128, (c0 + j) * W:
                              (c0 + j + 1) * W] for j in range(2)],
                        [xq_loc[i * 128:(i + 1) * 128, (c0 + j) * W:
                                (c0 + j + 1) * W] for j in range(2)],
                        c1x, float(isc), store_eng=nc.scalar)

        def x_half_gather(h, xttb, eng):
            # 32 transposes into one big staging tile (no slot-waits), one
            # 4MB store, then the AllGather
            for k in range(NK1):
                eng.dma_start(
                    xttb[:, k, :],
                    xq_loc[h * 512:(h + 1) * 512, k * 128:(k + 1) * 128],
                    transpose=True)
            eng.dma_start(
                xqT_locs[h][:].rearrange("(k p) c -> p k c", p=128), xttb[:])
            nc.gpsimd.collective_compute(
                "AllGather", OP.bypass, replica_groups=RG,
                ins=[xqT_locs[h][:].opt()], outs=[xqT_fulls[h][:].opt()])

        def w1_chunk_pairs(cc):
            # one 512-row chunk of w1 as a list of 8 pair-thunks
            thunks = []
            for rr in range(4):
                j = cc * 4 + rr
                for cp in range(2):
                    c0 = cp * 2
                    thunks.append((lambda j=j, c0=c0: quant_pair(
                        [w1_sh[j * 128:(j + 1) * 128, (c0 + u) * W:
                               (c0 + u + 1) * W] for u in range(2)],
                        [w1q[j * 128:(j + 1) * 128, (c0 + u) * W:
                             (c0 + u + 1) * W] for u in range(2)],
                        rdw1[:], tsw1[:], store_eng=nc.sync)))
            return thunks

        def w2_pair(p):
            # pair p (0..31): w2 row-tile j = p // 1 ... 2 calls per row-tile
            j = p
            quant_pair(
                [w2_sh[j * 128:(j + 1) * 128, u * W:(u + 1) * W]
                 for u in range(2)],
                [w2q[j * 128:(j + 1) * 128, u * W:(u + 1) * W]
                 for u in range(2)],
                rdw2[:], tsw2[:], store_eng=nc.sync)

        # ---------------- M1 machinery ----------------
        m1ps_cm = tc.tile_pool(name="m1ps", bufs=8, space="PSUM")
        m1ps = m1ps_cm.__enter__()

        def m1_tile(t, w1Tt, colbase):
            rb, ci = t % 8, t // 8
            h, off = rb // 4, (rb % 4) * 128
            xb = m1x.tile([128, NK1, 128], BF16, tag="xb", name="xb")
            nc.sync.dma_start(
                xb[:],
                xqT_fulls[h][ci * D_IN:(ci + 1) * D_IN, off:off + 128]
                .rearrange("(k p) c -> p k c", p=128))
            for chain in range(2):
                ps = m1ps.tile([128, 512], FP32, tag="ps", name="ps")
                for k in range(NK1):
                    nc.tensor.matmul(
                        ps[:], lhsT=xb[:, k, :],
                        rhs=w1Tt[:, k, chain * 512:(chain + 1) * 512],
                        start=(k == 0), stop=(k == NK1 - 1))
                g = m1g.tile([128, 512], FP32, tag="g", name="g")
                nc.scalar.activation(g[:], ps[:], AF.Gelu, scale=s_h[:])
                nc.scalar.dma_start(
                    g_dram[t * 128:(t + 1) * 128,
                           colbase + chain * 512:colbase + (chain + 1) * 512],
                    g[:])

        # S0 order: AG-half-0 tiles first; S1 order: global
        s0_order = [ci * 8 + h * 4 + r
                    for h in range(2) for ci in range(NCORES) for r in range(4)]

        # ================= Phase 0 =================
        for i in range(64):
            scan_chunk(w1_sh, i, acc1, 4)
        sam1 = allreduce_amax(acc1, 0, s1loc, s1sh, "sam1")
        x_half_quant(0)
        nc.vector.tensor_scalar(tsw1[:], sam1[:], inv2688, None, OP.mult)
        dw1 = singles.tile([128, 1], FP32, tag="dw1", name="dw1")
        nc.vector.tensor_scalar(dw1[:], tsw1[:], 6.0, None, OP.mult)
        nc.vector.reciprocal(rdw1[:], dw1[:])
        nc.vector.tensor_scalar(s_h[:], tsw1[:], float(isc), None, OP.mult)
        # gather staging tile lives on the right side; released before w2T_q1
        xttb_cm = tc.tile_pool(name="xttb", bufs=1, side="right")
        xttb_pool = xttb_cm.__enter__()
        xttb = xttb_pool.tile([128, NK1, 512], BF16, tag="xttb", name="xttb")
        x_half_gather(0, xttb, nc.sync)

        # w1 hi half (rows 1024-2047) -> w1T_hi window, then x half 1
        # (its gather rides the ACT ring so SP stays clear for S0's xb)
        for th in w1_chunk_pairs(3):
            th()
        for th in w1_chunk_pairs(2):
            th()
        x_half_quant(1)
        x_half_gather(1, xttb, nc.scalar)
        xttb_cm.__exit__(None, None, None)
        w2Tq_cms = [tc.tile_pool(name="w2T_q1", bufs=1, side="right")]
        w2Tq_pools = [w2Tq_cms[0].__enter__()]
        w2T_q = [w2Tq_pools[0].tile([128, NK2, 1024], BF16, tag="w2T_q1",
                                    name="w2T_q1")]
        w1T_hi_cm = tc.tile_pool(name="w1T_hi", bufs=1)
        w1T_hi_pool = w1T_hi_cm.__enter__()
        w1T_hi = w1T_hi_pool.tile([128, NK1, 1024], BF16, tag="w1T_hi",
                                  name="w1T_hi")
        for k in range(NK1):
            nc.sync.dma_start(w1T_hi[:, k, :],
                              w1q[1024:2048, k * 128:(k + 1) * 128],
                              transpose=True)

        # ================= M1 sweep S0 (cols 1024-2047) =================
        # w1 lo-half quant + the (dep-free) w2 amax scan ride inside S0
        ch1 = w1_chunk_pairs(1)
        ch0 = w1_chunk_pairs(0)

        for i, t in enumerate(s0_order[:32]):
            m1_tile(t, w1T_hi, 1024)
            if i % 4 == 3:
                ch1[i // 4]()
        for i, t in enumerate(s0_order[32:64]):
            m1_tile(t, w1T_hi, 1024)
            if i % 4 == 3:
                ch0[i // 4]()
            scan_chunk(w2_sh, 2 * i, acc2, 2)
            scan_chunk(w2_sh, 2 * i + 1, acc2, 2)

        # AR2 + scales
        sam2 = allreduce_amax(acc2, 1, s2loc, s2sh, "sam2")
        nc.vector.tensor_scalar(tsw2[:], sam2[:], inv2688, None, OP.mult)
        dw2 = singles.tile([128, 1], FP32, tag="dw2", name="dw2")
        nc.vector.tensor_scalar(dw2[:], tsw2[:], 6.0, None, OP.mult)
        nc.vector.reciprocal(rdw2[:], dw2[:])
        nc.vector.tensor_scalar(s_o[:], tsw2[:], float(hsc), None, OP.mult)

        # swap w1T window: hi -> lo (rows/cols 0-1023)
        w1T_hi_cm.__exit__(None, None, None)
        w1T_lo_cm = tc.tile_pool(name="w1T_lo", bufs=1)
        w1T_lo_pool = w1T_lo_cm.__enter__()
        w1T_lo = w1T_lo_pool.tile([128, NK1, 1024], BF16, tag="w1T_lo",
                                  name="w1T_lo")
        for k in range(NK1):
            nc.sync.dma_start(w1T_lo[:, k, :],
                              w1q[0:1024, k * 128:(k + 1) * 128],
                              transpose=True)

        # ================= M1 sweep S1 (cols 0-1023) =================
        for t in range(NBT):
            m1_tile(t, w1T_lo, 0)
            if t % 2 == 1:
                w2_pair(t // 2)
            if t == 33:
                # w2 rows 0-1023 quantized; fill w2T quarter 1
                for k in range(NK2):
                    nc.sync.dma_start(
                        w2T_q[0][:, k, :],
                        w2q[0:1024, k * 128:(k + 1) * 128], transpose=True)

        # ---------------- h-quant (chases M2, hq stays in SBUF) --------
        def h_tile(t):
            # hqT transposes all on ACT: keeps SP free for parts stores
            # (PSUM-drain path) so the M2 pipeline can't back up on them
            hqT = hqp.tile([128, NK2, 128], BF16, tag="hqT", name="hqT")
            outs = quant_pair(
                [g_dram[t * 128:(t + 1) * 128, u * W:(u + 1) * W]
                 for u in range(2)],
                None, c1h, float(hsc), signed=False)
            for u in range(2):
                for kk in range(8):
                    k = u * 8 + kk
                    nc.scalar.dma_start(hqT[:, k, :],
                                        outs[u][:, kk * 128:(kk + 1) * 128],
                                        transpose=True)
            return hqT

        hq_tiles = {}
        for tt in range(3):
            hq_tiles[tt] = h_tile(tt)

        # ================= Phase 2 (M2) =================
        m1ps_cm.__exit__(None, None, None)
        w1T_lo_cm.__exit__(None, None, None)
        m1g_cm.__exit__(None, None, None)
        m1x_cm.__exit__(None, None, None)
        for qi in range(1, 4):
            cm = tc.tile_pool(name=f"w2T_q{qi + 1}", bufs=1, side="right")
            w2Tq_cms.append(cm)
            pool = cm.__enter__()
            w2Tq_pools.append(pool)
            w2T_q.append(pool.tile([128, NK2, 1024], BF16,
                                   tag=f"w2T_q{qi + 1}", name=f"w2T_q{qi + 1}"))
            for k in range(NK2):
                nc.sync.dma_start(
                    w2T_q[qi][:, k, :],
                    w2q[qi * 1024:(qi + 1) * 1024, k * 128:(k + 1) * 128],
                    transpose=True)

        m2ps_cm = tc.tile_pool(name="m2ps", bufs=2, space="PSUM")
        m2ps = m2ps_cm.__enter__()

        for t in range(NBT):
            hqT = hq_tiles.pop(t)
            c, crow = t // 4, (t % 4) * 128
            for half in range(2):
                ps = m2ps.tile([128, 2048], FP32, tag="ps2", name="ps2")
                for k in range(NK2):
                    for u in range(4):
                        j = half * 4 + u
                        nc.tensor.matmul(
                            ps[:, u * 512:(u + 1) * 512],
                            lhsT=hqT[:, k, :],
                            rhs=w2T_q[j // 2][:, k,
                                              (j % 2) * 512:(j % 2 + 1) * 512],
                            start=(k == 0), stop=(k == NK2 - 1))
                for u in range(4):
                    ot = qp_src.tile([128, 512], BF16, tag="ot", name="ot",
                                     bufs=3)
                    nc.scalar.activation(ot[:], ps[:, u * 512:(u + 1) * 512],
                                         AF.Copy, scale=s_o[:])
                    nc.sync.dma_start(
                        parts[c][crow:crow + 128,
                                 half * 2048 + u * 512:
                                 half * 2048 + (u + 1) * 512], ot[:])
            if t % 4 == 3:
                nc.gpsimd.collective_compute(
                    "ReduceScatter", OP.add, replica_groups=RG,
                    ins=[parts[c][:].opt()], outs=[rsouts[c][:].opt()])
                nc.gpsimd.dma_start(
                    out_sh[c * RSOUT:(c + 1) * RSOUT, :], rsouts[c][:])
            if t + 3 < NBT:
                hq_tiles[t + 3] = h_tile(t + 3)

        m2ps_cm.__exit__(None, None, None)
        for cm in reversed(w2Tq_cms):
            cm.__exit__(None, None, None)
    nc.compile()
    return nc


def _get_built(isc, hsc):
    key = (float(isc), float(hsc))
    if key not in _BUILT:
        _BUILT[key] = _build(float(isc), float(hsc))
    return _BUILT[key]


def run(x, w1, w2, input_scale, hidden_scale, trace=False):
    from concourse import bass_utils
    isc = float(np.asarray(input_scale).reshape(-1)[0])
    hsc = float(np.asarray(hidden_scale).reshape(-1)[0])
    nc = _get_built(isc, hsc)
    x = np.ascontiguousarray(x, dtype=np.float32)
    w1 = np.ascontiguousarray(w1, dtype=np.float32)
    w2 = np.ascontiguousarray(w2, dtype=np.float32)
    in_maps = []
    for c in range(NCORES):
        in_maps.append({
            "x_sh": x[c * BSH:(c + 1) * BSH, :],
            "w1_sh": np.ascontiguousarray(w1[c * HSH:(c + 1) * HSH, :]),
            "w2_sh": np.ascontiguousarray(w2[:, c * HSH:(c + 1) * HSH]),
        })
    res = bass_utils.run_bass_kernel_spmd(
        nc, in_maps, core_ids=list(range(NCORES)), trace=trace)
    out = np.empty((B, D_OUT), dtype=np.float32)
    for r in range(NCORES):
        o = res.results[r]["out_sh"]
        for c in range(RSCH):
            out[c * RSROWS + r * RSOUT:c * RSROWS + (r + 1) * RSOUT, :] = \
                o[c * RSOUT:(c + 1) * RSOUT, :]
    return out, res


def kernel(x, w1, w2, input_scale, hidden_scale):
    out, _ = run(x, w1, w2, input_scale, hidden_scale, trace=False)
    return out


# revision 37
# speedup vs baseline: 1.1698x; 1.1698x over previous
"""NVFP4-fake-quant MLP (x@w1.T -> gelu -> @w2.T) on 8 trn2 NeuronCores.

Sharding (megatron tensor-parallel on the hidden dim):
  core c holds w1 rows [c*2048:(c+1)*2048], w2 cols [c*2048:(c+1)*2048],
  and x rows [c*1024:(c+1)*1024] (for distributed x-quantization).

Exact quantization:
  per-16-block e4m3 scales via exponent-mask + magic-number RNE;
  fp4 e2m1 rounding via 3-region clamp + magic-round decomposition.
  e2m1_value * e4m3_blockscale has <= 6 mantissa bits -> stored EXACTLY in
  bf16, so the bf16 matmuls reproduce the f32 reference; per-tensor scales
  are folded into the PSUM->SBUF copies (gelu input scale / output scale).

v5 schedule -- quant calls run PAIRED (software-pipelined across the
Vector/Scalar ping-pong), PE starts ~0.6ms in and stays dense:
  scans->AR1 first; w1 quantizes hi-half (rows 1024-2047) then lo-half,
  each into a 64KB w1T window (never 128KB resident).  x quantizes in
  row-halves with per-half xqT AllGathers.  M1 = two 1024-wide column
  sweeps (hi then lo); gelu drains spill g to DRAM f32.  w2 amax/AR2 +
  full w2 quant ride Vector slack inside the sweeps.  At M1 end the w1T
  window is swapped for right-side w2T quarter pools; M2 is one global
  b-tile pass; h-quant pairs chase M2 two tiles ahead, hq stays in SBUF
  and is transposed SBUF->SBUF straight into the stationary hqT tiles.
  ReduceScatter fires per 512-row chunk; f32 cast-stores chase it.
DMA placement: ACT ring = dep-free loads + x-side stores/transposes +
  g stores (self-paced by ACT compute); SP ring = weight stores, w1T/w2T
  transposes, xb loads, parts stores; SWDGE = collectives + AR staging +
  output casts.  Emission order per ring matches execution order.
"""
import os
import sys
import numpy as np

if "/opt/trn_rl_repo" not in sys.path:
    sys.path.insert(0, "/opt/trn_rl_repo")

f32 = np.float32

B, D_IN, HID, D_OUT = 8192, 4096, 16384, 4096
NCORES = 8
BSH = B // NCORES          # 1024 x-rows quantized per core
HSH = HID // NCORES        # 2048 hidden units per core
NBT = B // 128             # 64 b-tiles
RSCH = 16                  # reduce-scatter chunks
RSROWS = B // RSCH         # 512 rows per RS chunk
RSOUT = RSROWS // NCORES   # 64 rows per core per chunk
NK1 = D_IN // 128          # 32 k-tiles, first matmul
NK2 = HSH // 128           # 16 k-tiles, second matmul

# magic round-to-nearest-even constants (f32-exact)
C_HALF = float(f32(1.5 * 2 ** 22))       # grid 0.5
C_1 = float(f32(1.5 * 2 ** 23))          # grid 1
C_1B = float(f32(1.5 * 2 ** 23 + 2.0))   # C_1 + 2
C_2 = float(f32(1.5 * 2 ** 24))          # grid 2
C_2B = float(f32(1.5 * 2 ** 24 + 4.0))   # C_2 + 4
E4M3_MAGIC = float(f32(1.5 * 2 ** 20))   # * 2^e -> magic const for step 2^(e-3)
EXPMASK = 0x7F800000
SIGNMASK = 0x80000000
ONEBITS = 0x3F800000

_BUILT = {}


def _build(isc, hsc):
    from contextlib import ExitStack
    import concourse.bass as bass
    import concourse.tile as tile
    from concourse import bacc, mybir

    OP = mybir.AluOpType
    AF = mybir.ActivationFunctionType
    U32 = mybir.dt.uint32
    FP32 = mybir.dt.float32
    BF16 = mybir.dt.bfloat16

    c1x = float(f32(1.0) / (f32(6.0) * f32(isc)))
    c1h = float(f32(1.0) / (f32(6.0) * f32(hsc)))
    inv2688 = float(f32(1.0) / f32(2688.0))
    RG = [list(range(NCORES))]

    nc = bacc.Bacc("TRN2", target_bir_lowering=False, debug=False,
                   num_devices=NCORES)
    x_sh = nc.dram_tensor("x_sh", [BSH, D_IN], FP32, kind="ExternalInput").ap()
    w1_sh = nc.dram_tensor("w1_sh", [HSH, D_IN], FP32, kind="ExternalInput").ap()
    w2_sh = nc.dram_tensor("w2_sh", [D_OUT, HSH], FP32, kind="ExternalInput").ap()
    out_sh = nc.dram_tensor("out_sh", [BSH, D_OUT], FP32, kind="ExternalOutput").ap()

    with tile.TileContext(nc) as tc, ExitStack() as top:
        dram = top.enter_context(tc.tile_pool(name="dram", bufs=1, space="DRAM"))
        amax_stage = dram.tile([128, 2], FP32, tag="amax_stage", name="amax_stage")
        s1loc = dram.tile([1, 1], FP32, tag="s1loc", name="s1loc")
        s2loc = dram.tile([1, 1], FP32, tag="s2loc", name="s2loc")
        s1sh = dram.tile([1, 1], FP32, tag="s1sh", name="s1sh", addr_space="Shared")
        s2sh = dram.tile([1, 1], FP32, tag="s2sh", name="s2sh", addr_space="Shared")
        xq_loc = dram.tile([BSH, D_IN], BF16, tag="xq_loc", name="xq_loc")
        xqT_locs = [dram.tile([D_IN, 512], BF16, tag=f"xqT_loc{h}",
                              name=f"xqT_loc{h}") for h in range(2)]
        xqT_fulls = [dram.tile([NCORES * D_IN, 512], BF16,
                               tag=f"xqT_full{h}", name=f"xqT_full{h}",
                               addr_space="Shared") for h in range(2)]
        w1q = dram.tile([HSH, D_IN], BF16, tag="w1q", name="w1q")
        w2q = dram.tile([D_OUT, HSH], BF16, tag="w2q", name="w2q")
        g_dram = dram.tile([B, HSH], FP32, tag="g_dram", name="g_dram")
        hq = dram.tile([B, HSH], BF16, tag="hq", name="hq")
        parts = [dram.tile([RSROWS, D_OUT], BF16, name=f"part{c}", tag=f"part{c}")
                 for c in range(RSCH)]
        rsouts = [dram.tile([RSOUT, D_OUT], BF16, name=f"rsout{c}",
                            tag=f"rsout{c}")
                  for c in range(RSCH)]

        singles = top.enter_context(tc.tile_pool(name="singles", bufs=1))
        biases = {}
        for nm, val in [("ch", C_HALF), ("nch", -C_HALF),
                        ("c1", C_1), ("nc1b", -C_1B),
                        ("c2", C_2), ("nc2b", -C_2B)]:
            bt = singles.tile([128, 1], FP32, tag=f"bias_{nm}", name=f"bias_{nm}")
            nc.vector.memset(bt[:], val)
            biases[nm] = bt
        acc1 = singles.tile([128, 1], FP32, tag="acc1", name="acc1")
        acc2 = singles.tile([128, 1], FP32, tag="acc2", name="acc2")
        tsw1 = singles.tile([128, 1], FP32, tag="tsw1", name="tsw1")
        rdw1 = singles.tile([128, 1], FP32, tag="rdw1", name="rdw1")
        s_h = singles.tile([128, 1], FP32, tag="s_h", name="s_h")
        tsw2 = singles.tile([128, 1], FP32, tag="tsw2", name="tsw2")
        rdw2 = singles.tile([128, 1], FP32, tag="rdw2", name="rdw2")
        s_o = singles.tile([128, 1], FP32, tag="s_o", name="s_o")

        qp_src = top.enter_context(tc.tile_pool(name="qp_src", bufs=3))
        qp_f = top.enter_context(tc.tile_pool(name="qp_f", bufs=2))
        qp_b = top.enter_context(tc.tile_pool(name="qp_b", bufs=2))
        qp_n = top.enter_context(tc.tile_pool(name="qp_n", bufs=2))
        hqp = top.enter_context(tc.tile_pool(name="hqp", bufs=4))
        # m1x/m1g are released at M1 end (before w2T quarters 2-4 allocate)
        m1x_cm = tc.tile_pool(name="m1x", bufs=3)
        m1x = m1x_cm.__enter__()
        m1g_cm = tc.tile_pool(name="m1g", bufs=3)
        m1g = m1g_cm.__enter__()

        # ---------------- paired quant pipeline ----------------
        W = 1024
        NB = W // 16

        def q_load(src_slice):
            st = qp_src.tile([128, W], FP32, tag="xt", name="q_in")
            nc.scalar.dma_start(st[:], src_slice)
            return st

        def q_head(st, c1, effmul, signed):
            s = {"src": st, "signed": signed}
            if signed:
                absv = qp_f.tile([128, W], FP32, tag="q_absv", name="q_absv")
                nc.scalar.activation(absv[:], st[:], AF.Abs)
                s["mag"] = absv
            else:
                s["mag"] = st
            amax = qp_n.tile([128, NB], FP32, tag="q_amax", name="q_amax")
            nc.vector.tensor_reduce(
                amax[:], s["mag"][:].rearrange("p (nb b) -> p nb b", b=16),
                axis=mybir.AxisListType.X, op=OP.max,
                apply_absolute_value=(None if signed else True))
            vq = qp_n.tile([128, NB], FP32, tag="q_vq", name="q_vq")
            nc.vector.tensor_scalar(vq[:], amax[:], c1, None, OP.mult)
            scq = qp_n.tile([128, NB], FP32, tag="q_amax", name="q_scq")
            nc.vector.tensor_scalar(scq[:].bitcast(U32), vq[:].bitcast(U32),
                                    EXPMASK, None, OP.bitwise_and)
            cb = qp_n.tile([128, NB], FP32, tag="q_cb", name="q_cb")
            nc.vector.tensor_scalar(cb[:], scq[:], E4M3_MAGIC, None, OP.mult)
            t4 = qp_n.tile([128, NB], FP32, tag="q_t4", name="q_t4")
            nc.vector.tensor_tensor(t4[:], vq[:], cb[:], OP.add)
            bs = qp_n.tile([128, NB], FP32, tag="q_vq", name="q_bs")
            nc.vector.tensor_tensor(bs[:], t4[:], cb[:], OP.subtract)
            bs16 = qp_n.tile([128, NB], BF16, tag="q_bs16", name="q_bs16")
            nc.vector.tensor_scalar(bs16[:], bs[:], 2.0 ** -6, None, OP.max)
            eff = qp_n.tile([128, NB], FP32, tag="q_t4", name="q_eff")
            nc.vector.tensor_scalar(eff[:], bs[:], 2.0 ** -6, effmul,
                                    OP.max, OP.mult)
            rec = qp_n.tile([128, NB], FP32, tag="q_rec", name="q_rec")
            nc.vector.reciprocal(rec[:], eff[:])
            r = qp_f.tile([128, W], FP32, tag="q_r", name="q_r")
            nc.vector.tensor_tensor(
                r[:].rearrange("p (nb b) -> p nb b", b=16),
                s["mag"][:].rearrange("p (nb b) -> p nb b", b=16),
                rec[:, :, None].to_broadcast([128, NB, 16]), OP.mult)
            m1 = qp_f.tile([128, W], FP32, tag="q_absv", name="q_m1")
            nc.vector.tensor_scalar(m1[:], r[:], 2.0, None, OP.min)
            m3 = qp_f.tile([128, W], FP32, tag="q_m23", name="q_m3")
            nc.vector.tensor_scalar(m3[:], r[:], 4.0, 6.0, OP.max, OP.min)
            m2 = qp_f.tile([128, W], FP32, tag="q_m23", name="q_m2")
            nc.vector.tensor_scalar(m2[:], r[:], 2.0, 4.0, OP.max, OP.min)
            s.update(bs16=bs16, r=r, m1=m1, m2=m2, m3=m3)
            return s

        def q_acts(s):
            m1, m2, m3 = s["m1"], s["m2"], s["m3"]
            nc.scalar.activation(m3[:], m3[:], AF.Identity, bias=biases["c2"][:])
            s3 = qp_b.tile([128, W], BF16, tag="q_s3", name="q_s3")
            nc.scalar.activation(s3[:], m3[:], AF.Identity, bias=biases["nc2b"][:])
            nc.scalar.activation(m2[:], m2[:], AF.Identity, bias=biases["c1"][:])
            s2 = qp_b.tile([128, W], BF16, tag="q_s2", name="q_s2")
            nc.scalar.activation(s2[:], m2[:], AF.Identity, bias=biases["nc1b"][:])
            nc.scalar.activation(m1[:], m1[:], AF.Identity, bias=biases["ch"][:])
            s1 = qp_b.tile([128, W], BF16, tag="q_s1", name="q_s1", bufs=2)
            nc.scalar.activation(s1[:], m1[:], AF.Identity, bias=biases["nch"][:])
            s.update(s1=s1, s2=s2, s3=s3)

        def q_tail(s, out_ap):
            q12 = qp_b.tile([128, W], BF16, tag="q_q12", name="q_q12", bufs=2)
            nc.vector.tensor_tensor(q12[:], s["s1"][:], s["s2"][:], OP.add)
            qq = qp_b.tile([128, W], BF16, tag="q_s2", name="q_qq")
            nc.vector.tensor_tensor(qq[:], q12[:], s["s3"][:], OP.add)
            bs16 = s["bs16"]
            if s["signed"]:
                qs = qp_b.tile([128, W], BF16, tag="q_s1", name="q_qs", bufs=2)
                nc.vector.tensor_tensor(
                    qs[:].rearrange("p (nb b) -> p nb b", b=16),
                    qq[:].rearrange("p (nb b) -> p nb b", b=16),
                    bs16[:, :, None].to_broadcast([128, NB, 16]), OP.mult)
                sgn = qp_f.tile([128, W], FP32, tag="q_r", name="q_sgn")
                nc.vector.tensor_scalar(sgn[:].bitcast(U32),
                                        s["src"][:].bitcast(U32),
                                        SIGNMASK, ONEBITS,
                                        OP.bitwise_and, OP.bitwise_or)
                nc.vector.tensor_tensor(out_ap, qs[:], sgn[:], OP.mult)
            else:
                nc.vector.tensor_tensor(
                    out_ap.rearrange("p (nb b) -> p nb b", b=16),
                    qq[:].rearrange("p (nb b) -> p nb b", b=16),
                    bs16[:, :, None].to_broadcast([128, NB, 16]), OP.mult)

        def quant_pair(srcs, dsts, c1, effmul, signed=True, store_eng=None):
            """Two software-pipelined quant calls.  srcs: 2 DRAM slices;
            dsts: 2 DRAM slices (stored via store_eng) or None (returns
            the xo SBUF tiles)."""
            sts = [q_load(sl) for sl in srcs]
            states = [q_head(st, c1, effmul, signed) for st in sts]
            for s in states:
                q_acts(s)
            outs = []
            for i, s in enumerate(states):
                ot = qp_src.tile([128, W], BF16, tag="xo", name="q_out", bufs=2)
                q_tail(s, ot[:])
                if dsts is not None:
                    store_eng.dma_start(dsts[i], ot[:])
                outs.append(ot)
            return outs

        def scan_chunk(w_ap, i, acc, ncc):
            wt = qp_src.tile([128, W], FP32, tag="xt", name="scan_in")
            nc.scalar.dma_start(
                wt[:], w_ap[(i // ncc) * 128:(i // ncc + 1) * 128,
                            (i % ncc) * W:(i % ncc + 1) * W])
            am = qp_n.tile([128, 1], FP32, tag="am_w", name="am_w")
            nc.vector.tensor_reduce(am[:], wt[:], axis=mybir.AxisListType.X,
                                    op=OP.max, apply_absolute_value=True)
            if i == 0:
                nc.vector.tensor_copy(acc[:], am[:])
            else:
                nc.vector.tensor_tensor(acc[:], acc[:], am[:], OP.max)

        def allreduce_amax(acc, col, loc, sh, sam_name):
            # staging rides the SWDGE ring (gpsimd) so it never queues
            # behind PE-paced SP traffic
            nc.gpsimd.dma_start(amax_stage[:, col:col + 1], acc[:])
            rowv = singles.tile([1, 128], FP32, tag=f"rowv{col}",
                                name=f"rowv{col}")
            nc.gpsimd.dma_start(
                rowv[:],
                amax_stage[:, col:col + 1].rearrange("p c -> (p c)").unsqueeze(0))
            red = singles.tile([1, 1], FP32, tag=f"red{col}", name=f"red{col}")
            nc.vector.tensor_reduce(red[:], rowv[:],
                                    axis=mybir.AxisListType.X, op=OP.max)
            nc.gpsimd.dma_start(loc[:], red[:])
            nc.gpsimd.collective_compute(
                "AllReduce", OP.max, replica_groups=RG,
                ins=[loc[:].opt()], outs=[sh[:].opt()])
            sam = singles.tile([128, 1], FP32, tag=sam_name, name=sam_name)
            ap = sh[:]
            nc.gpsimd.dma_start(sam[:], bass.AP(
                tensor=ap.tensor, offset=ap.offset,
                ap=[[0, 128]] + list(ap.ap)[1:]))
            return sam

        # ---------------- phase-0 building blocks ----------------
        def x_half_quant(h):
            # quantize my x rows [h*512:(h+1)*512]; stores + transposes on
            # the ACT ring (self-paced with the quant chain)
            for i in range(h * 4, h * 4 + 4):
                for cp in range(2):
                    c0 = cp * 2
                    quant_pair(
                        [x_sh[i * 128:(i + 1) * 128, (c0 + j) * W:
                              (c0 + j + 1) * W] for j in range(2)],
                        [xq_loc[i * 128:(i + 1) * 128, (c0 + j) * W:
                                (c0 + j + 1) * W] for j in range(2)],
                        c1x, float(isc), store_eng=nc.scalar)

        def x_half_gather(h, xttb, eng):
            # 32 transposes into one big staging tile (no slot-waits), one
            # 4MB store, then the AllGather
            for k in range(NK1):
                eng.dma_start(
                    xttb[:, k, :],
                    xq_loc[h * 512:(h + 1) * 512, k * 128:(k + 1) * 128],
                    transpose=True)
            eng.dma_start(
                xqT_locs[h][:].rearrange("(k p) c -> p k c", p=128), xttb[:])
            nc.gpsimd.collective_compute(
                "AllGather", OP.bypass, replica_groups=RG,
                ins=[xqT_locs[h][:].opt()], outs=[xqT_fulls[h][:].opt()])

        def w1_chunk_pairs(cc):
            # one 512-row chunk of w1 as a list of 8 pair-thunks
            thunks = []
            for rr in range(4):
                j = cc * 4 + rr
                for cp in range(2):
                    c0 = cp * 2
                    thunks.append((lambda j=j, c0=c0: quant_pair(
                        [w1_sh[j * 128:(j + 1) * 128, (c0 + u) * W:
                               (c0 + u + 1) * W] for u in range(2)],
                        [w1q[j * 128:(j + 1) * 128, (c0 + u) * W:
                             (c0 + u + 1) * W] for u in range(2)],
                        rdw1[:], tsw1[:], store_eng=nc.sync)))
            return thunks

        def w2_pair(p):
            # pair p (0..31): w2 row-tile j = p // 1 ... 2 calls per row-tile
            j = p
            quant_pair(
                [w2_sh[j * 128:(j + 1) * 128, u * W:(u + 1) * W]
                 for u in range(2)],
                [w2q[j * 128:(j + 1) * 128, u * W:(u + 1) * W]
                 for u in range(2)],
                rdw2[:], tsw2[:], store_eng=nc.sync)

        # ---------------- M1 machinery ----------------
        m1ps_cm = tc.tile_pool(name="m1ps", bufs=8, space="PSUM")
        m1ps = m1ps_cm.__enter__()

        def m1_tile(t, w1Tt, colbase):
            rb, ci = t % 8, t // 8
            h, off = rb // 4, (rb % 4) * 128
            xb = m1x.tile([128, NK1, 128], BF16, tag="xb", name="xb")
            nc.sync.dma_start(
                xb[:],
                xqT_fulls[h][ci * D_IN:(ci + 1) * D_IN, off:off + 128]
                .rearrange("(k p) c -> p k c", p=128))
            for chain in range(2):
                ps = m1ps.tile([128, 512], FP32, tag="ps", name="ps")
                for k in range(NK1):
                    nc.tensor.matmul(
                        ps[:], lhsT=xb[:, k, :],
                        rhs=w1Tt[:, k, chain * 512:(chain + 1) * 512],
                        start=(k == 0), stop=(k == NK1 - 1))
                g = m1g.tile([128, 512], FP32, tag="g", name="g")
                nc.scalar.activation(g[:], ps[:], AF.Gelu, scale=s_h[:])
                nc.scalar.dma_start(
                    g_dram[t * 128:(t + 1) * 128,
                           colbase + chain * 512:colbase + (chain + 1) * 512],
                    g[:])

        # S0 order: AG-half-0 tiles first; S1 order: global
        s0_order = [ci * 8 + h * 4 + r
                    for h in range(2) for ci in range(NCORES) for r in range(4)]

        # ================= Phase 0 =================
        for i in range(64):
            scan_chunk(w1_sh, i, acc1, 4)
        sam1 = allreduce_amax(acc1, 0, s1loc, s1sh, "sam1")
        x_half_quant(0)
        nc.vector.tensor_scalar(tsw1[:], sam1[:], inv2688, None, OP.mult)
        dw1 = singles.tile([128, 1], FP32, tag="dw1", name="dw1")
        nc.vector.tensor_scalar(dw1[:], tsw1[:], 6.0, None, OP.mult)
        nc.vector.reciprocal(rdw1[:], dw1[:])
        nc.vector.tensor_scalar(s_h[:], tsw1[:], float(isc), None, OP.mult)
        # gather staging tile lives on the right side; released before w2T_q1
        xttb_cm = tc.tile_pool(name="xttb", bufs=1, side="right")
        xttb_pool = xttb_cm.__enter__()
        xttb = xttb_pool.tile([128, NK1, 512], BF16, tag="xttb", name="xttb")

        # w1 hi chunk 3 first; gather-h0 emitted AFTER it so its SP burst
        # doesn't block ch3's w1q stores (AG-h0 has ~300us of slack).
        for th in w1_chunk_pairs(3):
            th()
        x_half_gather(0, xttb, nc.sync)
        # x half 1 next (early AllGather), its gather on the ACT ring;
        # then ch2 completes the w1 hi window.
        x_half_quant(1)
        x_half_gather(1, xttb, nc.scalar)
        for th in w1_chunk_pairs(2):
            th()
        xttb_cm.__exit__(None, None, None)
        w2Tq_cms = [tc.tile_pool(name="w2T_q1", bufs=1, side="right")]
        w2Tq_pools = [w2Tq_cms[0].__enter__()]
        w2T_q = [w2Tq_pools[0].tile([128, NK2, 1024], BF16, tag="w2T_q1",
                                    name="w2T_q1")]
        w1T_hi_cm = tc.tile_pool(name="w1T_hi", bufs=1)
        w1T_hi_pool = w1T_hi_cm.__enter__()
        w1T_hi = w1T_hi_pool.tile([128, NK1, 1024], BF16, tag="w1T_hi",
                                  name="w1T_hi")
        for k in range(NK1):
            nc.sync.dma_start(w1T_hi[:, k, :],
                              w1q[1024:2048, k * 128:(k + 1) * 128],
                              transpose=True)

        # ================= M1 sweep S0 (cols 1024-2047) =================
        # w1 lo-half quant + the (dep-free) w2 amax scan ride inside S0
        ch1 = w1_chunk_pairs(1)
        ch0 = w1_chunk_pairs(0)

        for i, t in enumerate(s0_order[:32]):
            m1_tile(t, w1T_hi, 1024)
            if i % 4 == 3:
                ch1[i // 4]()
        for i, t in enumerate(s0_order[32:64]):
            m1_tile(t, w1T_hi, 1024)
            if i % 4 == 3:
                ch0[i // 4]()
            scan_chunk(w2_sh, 2 * i, acc2, 2)
            scan_chunk(w2_sh, 2 * i + 1, acc2, 2)

        # AR2 + scales
        sam2 = allreduce_amax(acc2, 1, s2loc, s2sh, "sam2")
        nc.vector.tensor_scalar(tsw2[:], sam2[:], inv2688, None, OP.mult)
        dw2 = singles.tile([128, 1], FP32, tag="dw2", name="dw2")
        nc.vector.tensor_scalar(dw2[:], tsw2[:], 6.0, None, OP.mult)
        nc.vector.reciprocal(rdw2[:], dw2[:])
        nc.vector.tensor_scalar(s_o[:], tsw2[:], float(hsc), None, OP.mult)

        # swap w1T window: hi -> lo (rows/cols 0-1023)
        w1T_hi_cm.__exit__(None, None, None)
        w1T_lo_cm = tc.tile_pool(name="w1T_lo", bufs=1)
        w1T_lo_pool = w1T_lo_cm.__enter__()
        w1T_lo = w1T_lo_pool.tile([128, NK1, 1024], BF16, tag="w1T_lo",
                                  name="w1T_lo")
        for k in range(NK1):
            nc.sync.dma_start(w1T_lo[:, k, :],
                              w1q[0:1024, k * 128:(k + 1) * 128],
                              transpose=True)

        # ================= M1 sweep S1 (cols 0-1023) =================
        for t in range(NBT):
            m1_tile(t, w1T_lo, 0)
            if t % 2 == 1:
                w2_pair(t // 2)
            if t == 33:
                # w2 rows 0-1023 quantized; fill w2T quarter 1
                for k in range(NK2):
                    nc.sync.dma_start(
                        w2T_q[0][:, k, :],
                        w2q[0:1024, k * 128:(k + 1) * 128], transpose=True)

        # ---------------- h-quant (chases M2 via DRAM hq) ----------------
        def h_group(g):
            # quantize tiles 2g, 2g+1 into DRAM hq, then one [256,*]
            # transposed group load per k-tile into the stationary hqT
            for ti in range(2):
                t = 2 * g + ti
                quant_pair(
                    [g_dram[t * 128:(t + 1) * 128, u * W:(u + 1) * W]
                     for u in range(2)],
                    [hq[t * 128:(t + 1) * 128, u * W:(u + 1) * W]
                     for u in range(2)],
                    c1h, float(hsc), signed=False, store_eng=nc.scalar)
            hqT = hqp.tile([128, NK2, 256], BF16, tag="hqT", name="hqT", bufs=2)
            for k in range(NK2):
                nc.sync.dma_start(
                    hqT[:, k, :],
                    hq[g * 256:(g + 1) * 256, k * 128:(k + 1) * 128],
                    transpose=True)
            return hqT

        hq_groups = {0: h_group(0), 1: h_group(1)}

        # ================= Phase 2 (M2) =================
        m1ps_cm.__exit__(None, None, None)
        w1T_lo_cm.__exit__(None, None, None)
        m1g_cm.__exit__(None, None, None)
        m1x_cm.__exit__(None, None, None)
        for qi in range(1, 4):
            cm = tc.tile_pool(name=f"w2T_q{qi + 1}", bufs=1, side="right")
            w2Tq_cms.append(cm)
            pool = cm.__enter__()
            w2Tq_pools.append(pool)
            w2T_q.append(pool.tile([128, NK2, 1024], BF16,
                                   tag=f"w2T_q{qi + 1}", name=f"w2T_q{qi + 1}"))
            for k in range(NK2):
                nc.sync.dma_start(
                    w2T_q[qi][:, k, :],
                    w2q[qi * 1024:(qi + 1) * 1024, k * 128:(k + 1) * 128],
                    transpose=True)

        m2ps_cm = tc.tile_pool(name="m2ps", bufs=2, space="PSUM")
        m2ps = m2ps_cm.__enter__()

        for g in range(NBT // 2):
            hqT = hq_groups.pop(g)
            for ti in range(2):
                t = 2 * g + ti
                c, crow = t // 4, (t % 4) * 128
                for half in range(2):
                    ps = m2ps.tile([128, 2048], FP32, tag="ps2", name="ps2")
                    for k in range(NK2):
                        for u in range(4):
                            j = half * 4 + u
                            nc.tensor.matmul(
                                ps[:, u * 512:(u + 1) * 512],
                                lhsT=hqT[:, k, ti * 128:(ti + 1) * 128],
                                rhs=w2T_q[j // 2][:, k, (j % 2) * 512:
                                                  (j % 2 + 1) * 512],
                                start=(k == 0), stop=(k == NK2 - 1))
                    for u in range(4):
                        ot = qp_src.tile([128, 512], BF16, tag="ot",
                                         name="ot", bufs=3)
                        nc.scalar.activation(ot[:],
                                             ps[:, u * 512:(u + 1) * 512],
                                             AF.Copy, scale=s_o[:])
                        nc.sync.dma_start(
                            parts[c][crow:crow + 128,
                                     half * 2048 + u * 512:
                                     half * 2048 + (u + 1) * 512], ot[:])
                if t % 4 == 3:
                    nc.gpsimd.collective_compute(
                        "ReduceScatter", OP.add, replica_groups=RG,
                        ins=[parts[c][:].opt()], outs=[rsouts[c][:].opt()])
                    nc.gpsimd.dma_start(
                        out_sh[c * RSOUT:(c + 1) * RSOUT, :], rsouts[c][:])
            if g + 2 < NBT // 2:
                hq_groups[g + 2] = h_group(g + 2)

        m2ps_cm.__exit__(None, None, None)
        for cm in reversed(w2Tq_cms):
            cm.__exit__(None, None, None)
    nc.compile()
    return nc


def _get_built(isc, hsc):
    key = (float(isc), float(hsc))
    if key not in _BUILT:
        _BUILT[key] = _build(float(isc), float(hsc))
    return _BUILT[key]


def run(x, w1, w2, input_scale, hidden_scale, trace=False):
    from concourse import bass_utils
    isc = float(np.asarray(input_scale).reshape(-1)[0])
    hsc = float(np.asarray(hidden_scale).reshape(-1)[0])
    nc = _get_built(isc, hsc)
    x = np.ascontiguousarray(x, dtype=np.float32)
    w1 = np.ascontiguousarray(w1, dtype=np.float32)
    w2 = np.ascontiguousarray(w2, dtype=np.float32)
    in_maps = []
    for c in range(NCORES):
        in_maps.append({
            "x_sh": x[c * BSH:(c + 1) * BSH, :],
            "w1_sh": np.ascontiguousarray(w1[c * HSH:(c + 1) * HSH, :]),
            "w2_sh": np.ascontiguousarray(w2[:, c * HSH:(c + 1) * HSH]),
        })
    res = bass_utils.run_bass_kernel_spmd(
        nc, in_maps, core_ids=list(range(NCORES)), trace=trace)
    out = np.empty((B, D_OUT), dtype=np.float32)
    for r in range(NCORES):
        o = res.results[r]["out_sh"]
        for c in range(RSCH):
            out[c * RSROWS + r * RSOUT:c * RSROWS + (r + 1) * RSOUT, :] = \
                o[c * RSOUT:(c + 1) * RSOUT, :]
    return out, res


def kernel(x, w1, w2, input_scale, hidden_scale):
    out, _ = run(x, w1, w2, input_scale, hidden_scale, trace=False)
    return out
